# revision 14
# baseline (speedup 1.0000x reference)
"""Cube padding kernel for Trainium2 (Bass/Tile), 8-core SPMD.

Op: x [B=4, 6, C=64, H=128, W=128] f32 -> out [B, 6, C, H+2P, W+2P], P=2.
Each face's pad ring is gathered from neighboring faces (with flips /
transposes per the cube-net layout) and corners replicate the top/bottom
strip edge values.

Sharding: channel-parallel. C=64 is split into 8 chunks of 8 channels; every
core holds all 6 faces for its channel slice, so no cross-core traffic.

Per-core dataflow (per batch b):
  - DMA each face plane HBM -> SBUF tile O[f][h, c, 2:130] (the interior of
    the output rows; border columns are filled in place so the store back to
    HBM is one 528B-contiguous run per row).
  - PE transposes (exact pass-through via identity matmul) produce the
    transposed border data needed by the left/right/top/down faces.
  - DVE fills left/right border columns of O from other faces' SBUF tiles or
    PSUM transposes (partition-preserving copies).
  - Top/bottom 2-row strips (+corners) are assembled in small [16,136] tiles
    (partition = (r, c)) via DMA / DVE-from-PSUM + corner broadcasts, then
    stored as 528B rows.

All SBUF/PSUM access patterns keep the partition index in the leading AP dim
with stride == the tile's free-row size (plain partition-range slices): both
the simulator's shadow tracker and the DMA addr64 lowering are only reliable
for that form.
"""

import numpy as np

import concourse.bacc as bacc
import concourse.mybir as mybir
from concourse import tile
from concourse.bass_utils import run_bass_kernel_spmd

P = 2
B, F, C, H, W = 4, 6, 64, 128, 128
NCORES = 8
CL = C // NCORES  # channels per core
HO, WO = H + 2 * P, W + 2 * P  # 132, 132
TW = 136  # strip tile row = full 544B slot row (32B-aligned)
FP32 = mybir.dt.float32

# face indices (order of unpacking in the reference: fb, fd, ff, fl, fr, ft)
BACK, DOWN, FRONT, LEFT, RIGHT, TOP = range(6)

# --- strip source tables -----------------------------------------------------
# TOP strips fill output rows 0..1 (r=0 is row 0), BOT strips rows 130..131
# (r=0 is row 130). Kinds:
#   ('rows', face, [i0, i1])  strip row r <- face row i_r          (DMA)
#   ('ptc',  face, [w0, w1])  strip row r <- face column w_r, transposed on
#                             PE with columns pre-ordered (w0, w1) so the
#                             PSUM result is evacuated by an identity-
#                             partition DVE copy.
TOP_SRC = {
    BACK: ("rows", TOP, [1, 0]),
    DOWN: ("rows", FRONT, [126, 127]),
    FRONT: ("rows", TOP, [126, 127]),
    LEFT: ("ptc", TOP, [0, 1]),
    RIGHT: ("ptc", TOP, [127, 126]),
    TOP: ("rows", BACK, [1, 0]),
}
BOT_SRC = {
    BACK: ("rows", DOWN, [127, 126]),
    DOWN: ("rows", BACK, [127, 126]),
    FRONT: ("rows", DOWN, [0, 1]),
    LEFT: ("ptc", DOWN, [1, 0]),
    RIGHT: ("ptc", DOWN, [126, 127]),
    TOP: ("rows", FRONT, [0, 1]),
}
# LFT fills O cols 0..1 (k=0 is col 0), RGT fills O cols 130..131. Kinds:
#   ('cols', face, [w0, w1])  col k <- face col w_k     (DVE from O[face])
#   ('pt',   face, [j0, j1])  col k <- PT[face][:, c, j_k] (DVE from PSUM)
LFT_SRC = {
    BACK: ("cols", RIGHT, [126, 127]),
    DOWN: ("pt", LEFT, [127, 126]),
    FRONT: ("cols", LEFT, [126, 127]),
    LEFT: ("cols", BACK, [126, 127]),
    RIGHT: ("cols", FRONT, [126, 127]),
    TOP: ("pt", LEFT, [0, 1]),
}
RGT_SRC = {
    BACK: ("cols", LEFT, [0, 1]),
    DOWN: ("pt", RIGHT, [126, 127]),
    FRONT: ("cols", RIGHT, [0, 1]),
    LEFT: ("cols", FRONT, [0, 1]),
    RIGHT: ("cols", BACK, [0, 1]),
    TOP: ("pt", RIGHT, [1, 0]),
}


def _pair_slice(idx):
    """Slice selecting [i0, i1] for adjacent pairs (ascending or descending)."""
    i0, i1 = idx
    assert abs(i1 - i0) == 1
    if i1 > i0:
        return slice(i0, i1 + 1)
    return slice(i0, (i1 - 1) if i1 > 0 else None, -1)


def build_kernel(nc, tc, xin, ident, yout):
    # Every SBUF tile gets a unique tag with bufs=1: the whole working set
    # (~130KB/partition) fits in SBUF, so no slot recycling is needed and the
    # scheduler gets maximal reordering freedom.
    with (
        tc.tile_pool(name="const", bufs=1) as const_pool,
        tc.tile_pool(name="io", bufs=1) as io_pool,
        tc.tile_pool(name="strips", bufs=1) as strip_pool,
        tc.tile_pool(name="psum", bufs=1, space="PSUM") as psum_pool,
    ):
        idt = const_pool.tile([128, 128], FP32, name="idt")
        nc.sync.dma_start(idt[:, :], ident[:, :])

        for b in range(B):
            # --- load all 6 face interiors into O tiles ---
            # The 8-col edge memsets cover the border cols' 32B sectors (the
            # sim's uninit tracker works at sector granularity) and overlap
            # the interior DMA region so program order is enforced via WAW.
            O = {}
            for f in range(F):
                O[f] = io_pool.tile(
                    [128, CL, WO], FP32, name=f"O{b}_{f}", tag=f"O{b}_{f}"
                )
                nc.gpsimd.memset(O[f][:, :, 0:8], 0.0)
                nc.gpsimd.memset(O[f][:, :, WO - 8 : WO], 0.0)
                nc.sync.dma_start(
                    O[f][:, :, P : P + W], xin[b, f].transpose((1, 0, 2))
                )

            # --- PE transposes ---
            # Full-plane transposes of LEFT/RIGHT faces (per channel):
            # PT[f][p, c, j] = face[c, j, p]
            PT = {}
            for f in (LEFT, RIGHT):
                PT[f] = psum_pool.tile(
                    [128, CL, 128], FP32, name=f"PT{b}_{f}", tag=f"PT{f}"
                )
                for c in range(CL):
                    nc.tensor.transpose(
                        PT[f][:, c, :], O[f][:, c, P : P + W], idt[:, :]
                    )

            # Single-column transposes for the 'ptc' strips: pts[:, i, :] is
            # [CL part = c, 128] = one transposed source column, in consumer
            # row order (PE matmul and DVE both require partition base 0, so
            # the strip rows live in the free dim of [CL, 2, TW] tiles).
            pts = psum_pool.tile([CL, 8, 128], FP32, name=f"pts{b}", tag="pts", bufs=2)
            ptc_out = {}
            ptc_specs = [
                (TOP_SRC, "Ttop", LEFT),
                (TOP_SRC, "Ttop", RIGHT),
                (BOT_SRC, "Tbot", LEFT),
                (BOT_SRC, "Tbot", RIGHT),
            ]
            for i, (table, sname, f) in enumerate(ptc_specs):
                _, src_f, wpair = table[f]
                outs = []
                for r in range(2):
                    col = P + wpair[r]
                    nc.tensor.transpose(
                        pts[:, 2 * i + r, :],
                        O[src_f][:, :, col : col + 1].squeeze(),
                        idt[:, :],
                    )
                    outs.append(pts[:, 2 * i + r, :])
                ptc_out[(sname, f)] = outs

            for f in range(F):
                # --- left/right border columns of O (DVE, partition-preserving) ---
                for dst_col, table in ((0, LFT_SRC), (W + P, RGT_SRC)):
                    kind, src_f, idx = table[f][0], table[f][1], table[f][2]
                    if kind == "cols":
                        src = O[src_f][:, :, P + idx[0] : P + idx[1] + 1]
                    else:  # 'pt'
                        src = PT[src_f][:, :, _pair_slice(idx)]
                    nc.vector.tensor_copy(O[f][:, :, dst_col : dst_col + 2], src)

                # --- top/bottom strips ---
                # 'rows' strips use [16, TW] tiles (partition = r*CL+c, rows
                # loaded by DMA which can remap partitions); 'ptc' strips use
                # [CL, 2, TW] tiles (partition = c, rows in the free dim, so
                # DVE can evacuate the PSUM transposes partition-preserving).
                strips = []
                for sname, table in (("Ttop", TOP_SRC), ("Tbot", BOT_SRC)):
                    spec = table[f]
                    if spec[0] == "rows":
                        Tfull = strip_pool.tile(
                            [16, TW], FP32,
                            name=f"{sname}{b}_{f}", tag=f"{sname}{b}_{f}",
                        )
                        nc.gpsimd.memset(Tfull[:, :], 0.0)
                        _, src_f, idx = spec
                        xv = xin[b, src_f].transpose((1, 0, 2))
                        if idx[1] > idx[0]:
                            nc.sync.dma_start(
                                Tfull[:, P : P + W], xv[idx[0] : idx[1] + 1]
                            )
                        else:
                            # descending row pair: DMA APs reject negative
                            # partition steps, so emit one DMA per row
                            for r in range(2):
                                nc.sync.dma_start(
                                    Tfull[r * CL : (r + 1) * CL, P : P + W],
                                    xv[idx[r] : idx[r] + 1],
                                )
                        nc.vector.tensor_copy(
                            Tfull[:, 0:P], Tfull[:, P : P + 1].broadcast_to((16, P))
                        )
                        nc.vector.tensor_copy(
                            Tfull[:, P + W : P + W + P],
                            Tfull[:, P + W - 1 : P + W].broadcast_to((16, P)),
                        )
                        strips.append(Tfull[:, :WO])
                    else:
                        Trc = strip_pool.tile(
                            [CL, 2, TW], FP32,
                            name=f"{sname}{b}_{f}", tag=f"{sname}{b}_{f}",
                        )
                        nc.gpsimd.memset(Trc[:, :, :], 0.0)
                        for r in range(2):
                            nc.vector.tensor_copy(
                                Trc[:, r, P : P + W], ptc_out[(sname, f)][r]
                            )
                        nc.vector.tensor_copy(
                            Trc[:, :, 0:P],
                            Trc[:, :, P : P + 1].broadcast_to((CL, 2, P)),
                        )
                        nc.vector.tensor_copy(
                            Trc[:, :, P + W : P + W + P],
                            Trc[:, :, P + W - 1 : P + W].broadcast_to((CL, 2, P)),
                        )
                        strips.append(Trc[:, :, :WO])

                # --- stores ---
                nc.sync.dma_start(
                    yout[b, f][:, P : P + H, :].transpose((1, 0, 2)), O[f][:, :, :]
                )
                for sname, Ts, rows0 in (
                    ("Ttop", strips[0], 0),
                    ("Tbot", strips[1], P + H),
                ):
                    dst = yout[b, f][:, rows0 : rows0 + P, :]
                    if Ts.ndim == 2:  # [16, WO]: partition (r, c) -> (r, c, w)
                        nc.sync.dma_start(dst.transpose((1, 0, 2)), Ts)
                    else:  # [CL, 2, WO]: partition c -> (c, r, w)
                        nc.sync.dma_start(dst, Ts)


def build_nc(debug=False, detect_races=True):
    nc = bacc.Bacc(
        "TRN2",
        target_bir_lowering=False,
        debug=debug,
        detect_race_conditions=detect_races,
    )
    xin = nc.dram_tensor("x", [B, F, CL, H, W], FP32, kind="ExternalInput")
    ident = nc.dram_tensor("ident", [128, 128], FP32, kind="ExternalInput")
    yout = nc.dram_tensor("y", [B, F, CL, HO, WO], FP32, kind="ExternalOutput")
    with tile.TileContext(nc) as tc:
        build_kernel(nc, tc, xin.ap(), ident.ap(), yout.ap())
    nc.compile()  # bacc passes (register allocation etc.) — required for NEFF
    return nc


_IDENT = np.eye(128, dtype=np.float32)


def kernel(x: np.ndarray) -> np.ndarray:
    x = np.asarray(x, dtype=np.float32)
    assert x.shape == (B, F, C, H, W), x.shape
    nc = build_nc()
    in_maps = [
        {
            "x": np.ascontiguousarray(x[:, :, k * CL : (k + 1) * CL]),
            "ident": _IDENT,
        }
        for k in range(NCORES)
    ]
    res = run_bass_kernel_spmd(nc, in_maps, core_ids=list(range(NCORES))).results
    return np.concatenate([res[k]["y"] for k in range(NCORES)], axis=2)
